# revision 20
# baseline (speedup 1.0000x reference)
"""DeepSeek-V2-Lite-style MoE layer on 8 Trainium2 NeuronCores.

Sharding: expert-parallel (8 experts/core) for the routed experts; the shared
MLP is tensor-parallel along its intermediate dim (256/core); the fp32 router
and grouped-top-k routing run replicated on every core. Each core gathers the
tokens routed to its local experts straight from (replicated) hidden_states,
runs the grouped GEMMs in float32r, combines locally with the routing weights
plus its shared-MLP partial, and a ReduceScatter sums partials across cores so
core c returns output rows [128c, 128(c+1)).

Config (DeepSeek-V2-Lite): T=1024 H=2048 E=64 K=6 I=1024 G=8 TG=3 C=256
"""
import numpy as np
from contextlib import ExitStack

import concourse.bass as bass
import concourse.tile as tile
from concourse import bacc
from concourse import mybir
from concourse import bass_utils

T, H, E, K, I, G, TG, C = 1024, 2048, 64, 6, 1024, 8, 3, 256
S = 2048                  # full shared intermediate (NS * I)
RSF = 2.5
NCORES = 8
EL = E // NCORES          # local experts per core
SL = S // NCORES          # local shared-intermediate slice per core
P = 128
NT = T // P               # token tiles
NKH = H // P              # K-tiles over H
NIT = I // P              # I-tiles
BIG = 1.0e30

F32 = mybir.dt.float32
F32R = mybir.dt.float32r
U32 = mybir.dt.uint32
U8 = mybir.dt.uint8
AX = mybir.AxisListType
ALU = mybir.AluOpType
ACTF = mybir.ActivationFunctionType


def build_program(use_collective=True):
    nc = bacc.Bacc("TRN2", target_bir_lowering=False, debug=False,
                   num_devices=NCORES)

    # ---- inputs (per-core; sliced/prepped on host)
    hs_r = nc.dram_tensor("hs_r", [T, H], F32R, kind="ExternalInput")
    hsT = nc.dram_tensor("hsT", [H, T], F32, kind="ExternalInput")
    gwT = nc.dram_tensor("gwT", [H, E], F32, kind="ExternalInput")
    bias_bc = nc.dram_tensor("bias_bc", [P, E], F32, kind="ExternalInput")
    u128 = nc.dram_tensor("u128", [P, P], F32, kind="ExternalInput")
    ones1 = nc.dram_tensor("ones1", [1, P], F32, kind="ExternalInput")
    onesc = nc.dram_tensor("onesc", [P, 1], F32, kind="ExternalInput")
    ident_r = nc.dram_tensor("ident_r", [P, P], F32R, kind="ExternalInput")
    w13_loc = nc.dram_tensor("w13_loc", [EL, H, 2 * I], F32R, kind="ExternalInput")
    w2_loc = nc.dram_tensor("w2_loc", [EL, I, H], F32R, kind="ExternalInput")
    sgu_loc = nc.dram_tensor("sgu_loc", [H, 2 * SL], F32, kind="ExternalInput")
    sd_loc = nc.dram_tensor("sd_loc", [SL, H], F32, kind="ExternalInput")

    out_t = nc.dram_tensor("out", [P if use_collective else T, H], F32,
                           kind="ExternalOutput")

    # ---- internal DRAM scratch
    table = nc.dram_tensor("table", [EL * C, 1], U32, kind="Internal")
    y_buf = nc.dram_tensor("y_buf", [EL * C, H], F32, kind="Internal")
    cc_in = nc.dram_tensor("cc_in", [T, H], F32, kind="Internal")
    if use_collective:
        cc_out = nc.dram_tensor("cc_out", [P, H], F32, kind="Internal")

    with tile.TileContext(nc) as tc, ExitStack() as ctx:
        cst = ctx.enter_context(tc.tile_pool(name="cst", bufs=1))
        rt = ctx.enter_context(tc.tile_pool(name="rt", bufs=2))
        rkeep = ctx.enter_context(tc.tile_pool(name="rkeep", bufs=1))
        # expert weight pools open for the whole kernel so their DMAs can
        # prefetch during routing/shared phases
        we_pool = ctx.enter_context(tc.tile_pool(name="we", bufs=3))
        w2_pool = ctx.enter_context(tc.tile_pool(name="w2", bufs=2))

        # ---------------- constants
        bias_t = cst.tile([P, E], F32, tag="bias")
        nc.sync.dma_start(bias_t[:], bias_bc[:])
        u_t = cst.tile([P, P], F32, tag="u128")
        nc.sync.dma_start(u_t[:], u128[:])
        ones_t = cst.tile([1, P], F32, tag="ones1")
        nc.sync.dma_start(ones_t[:], ones1[:])
        onesc_t = cst.tile([P, 1], F32, tag="onesc")
        nc.sync.dma_start(onesc_t[:], onesc[:])
        id_t = cst.tile([P, P], F32R, tag="ident")
        nc.sync.dma_start(id_t[:], ident_r[:])
        gw_t = cst.tile([P, NKH * E], F32, tag="gwT")   # [128, kt*64]
        nc.sync.dma_start(gw_t[:].rearrange("p (kt e) -> p kt e", kt=NKH),
                          gwT[:].rearrange("(kt p) e -> p kt e", p=P))
        iot_e = cst.tile([P, E], U32, tag="iote")
        nc.gpsimd.iota(iot_e[:], pattern=[[1, E]], base=0, channel_multiplier=0)
        # per-slot column index 0..C-1 repeated for the EL local experts
        c_iota = cst.tile([P, EL * C], F32, tag="c_iota")
        nc.gpsimd.iota(c_iota[:], pattern=[[0, EL], [1, C]], base=0,
                       channel_multiplier=0,
                       allow_small_or_imprecise_dtypes=True)
        off_sb = cst.tile([1, E], F32, tag="off")
        nc.vector.memset(off_sb[:], 0.0)

        # per-(token,k) combine data, resident until the combine phase
        wk_keep = rkeep.tile([P, NT * K], F32, tag="wk_keep")
        sl_keep = rkeep.tile([P, NT * K], U32, tag="sl_keep")

        # ============ phases R (routing) + S (shared MLP): own scratch pools
        with ExitStack() as rs_ctx:
            hst_pool = rs_ctx.enter_context(tc.tile_pool(name="hst", bufs=4))
            tbl_pool = rs_ctx.enter_context(tc.tile_pool(name="tblsb", bufs=1))
            r_ctx = rs_ctx.enter_context(ExitStack())
            lg_ps_pool = r_ctx.enter_context(
                tc.tile_pool(name="lgps", bufs=2, space="PSUM"))
            pos_ps_pool = r_ctx.enter_context(
                tc.tile_pool(name="posps", bufs=1, space="PSUM"))
            cs_ps_pool = r_ctx.enter_context(
                tc.tile_pool(name="csps", bufs=1, space="PSUM"))

            # -------- router matmul: stream [128,128] hsT slices per (tt, kt)
            logits_list = []
            for tt in range(NT):
                lg = lg_ps_pool.tile([P, E], F32, tag="logits")
                for kt in range(NKH):
                    hv = hst_pool.tile([P, P], F32, tag="hstr")
                    nc.sync.dma_start(
                        hv[:], hsT[kt * P:(kt + 1) * P, tt * P:(tt + 1) * P])
                    nc.tensor.matmul(
                        lg[:], hv[:],
                        gw_t[:, kt * E:(kt + 1) * E],
                        start=(kt == 0), stop=(kt == NKH - 1))
                logits_list.append(lg)

            # -------- grouped top-k routing per token tile
            tbl_ps = cs_ps_pool.tile([1, EL * C], F32, tag="tblps")
            for it in range(NT):
                scores = rt.tile([P, E], F32, tag="scores")
                nc.scalar.activation(scores[:], logits_list[it][:],
                                     ACTF.Sigmoid)
                sc = rt.tile([P, E], F32, tag="sc")
                nc.vector.tensor_tensor(sc[:], scores[:], bias_t[:], ALU.add)

                grp = sc[:].rearrange("p (g e) -> p g e", g=G)
                m1 = rt.tile([P, G], F32, tag="m1")
                nc.vector.tensor_reduce(m1[:], grp, axis=AX.X, op=ALU.max)
                eq = rt.tile([P, E], F32, tag="eq")
                m1b = m1[:].rearrange("p (g o) -> p g o", o=1).broadcast_to((P, G, G))
                nc.vector.tensor_tensor(eq[:].rearrange("p (g e) -> p g e", g=G),
                                        grp, m1b, ALU.is_ge)
                pen = rt.tile([P, E], F32, tag="pen")
                nc.vector.tensor_scalar(pen[:], eq[:], -BIG, None, op0=ALU.mult)
                msk2 = rt.tile([P, E], F32, tag="msk2")
                nc.vector.tensor_tensor(msk2[:], sc[:], pen[:], ALU.add)
                m2 = rt.tile([P, G], F32, tag="m2")
                nc.vector.tensor_reduce(
                    m2[:], msk2[:].rearrange("p (g e) -> p g e", g=G),
                    axis=AX.X, op=ALU.max)
                g2 = rt.tile([P, G], F32, tag="g2")
                nc.vector.tensor_tensor(g2[:], m1[:], m2[:], ALU.add)

                gv8 = rt.tile([P, 8], F32, tag="gv8")
                gi8 = rt.tile([P, 8], U32, tag="gi8")
                nc.vector.max_with_indices(gv8[:], gi8[:], g2[:])
                gmask = rt.tile([P, G], F32, tag="gmask")
                nc.vector.tensor_tensor(gmask[:], g2[:],
                                        gv8[:, 2:3].broadcast_to((P, G)), ALU.is_ge)

                gm64 = rt.tile([P, E], U8, tag="gm64")
                gmb = gmask[:].rearrange("p (g o) -> p g o", o=1) \
                    .broadcast_to((P, G, G))
                nc.vector.tensor_copy(gm64[:].rearrange("p (g e) -> p g e", g=G),
                                      gmb)
                scm = rt.tile([P, E], F32, tag="scm")
                nc.vector.memset(scm[:], -BIG)
                nc.vector.copy_predicated(scm[:], gm64[:], sc[:])
                v8 = rt.tile([P, 8], F32, tag="v8")
                i8 = rt.tile([P, 8], U32, tag="i8")
                nc.vector.max_with_indices(v8[:], i8[:], scm[:])

                A = rt.tile([P, E], F32, tag="A")
                nc.vector.memset(A[:], 0.0)
                scok = rt.tile([P, K], F32, tag="scok")
                ohs = []
                for k in range(K):
                    oh = rt.tile([P, E], F32, tag=f"oh{k}")
                    ohs.append(oh)
                    nc.vector.tensor_tensor(oh[:], iot_e[:],
                                            i8[:, k:k + 1].broadcast_to((P, E)),
                                            ALU.is_equal)
                    nc.vector.tensor_tensor(A[:], A[:], oh[:], ALU.add)
                    tmp = rt.tile([P, E], F32, tag="ttmp")
                    nc.vector.tensor_tensor(tmp[:], scores[:], oh[:], ALU.mult)
                    nc.vector.tensor_reduce(scok[:, k:k + 1], tmp[:], axis=AX.X,
                                            op=ALU.add)

                pos_ps = pos_ps_pool.tile([P, E], F32, tag="posps")
                nc.tensor.matmul(pos_ps[:], u_t[:], A[:], start=True, stop=False)
                nc.tensor.matmul(pos_ps[:], ones_t[:], off_sb[:], start=False,
                                 stop=True)
                pos = rt.tile([P, E], F32, tag="pos")
                nc.vector.tensor_copy(pos[:], pos_ps[:])
                cs_ps = cs_ps_pool.tile([1, E], F32, tag="csps")
                nc.tensor.matmul(cs_ps[:], onesc_t[:], A[:], start=True,
                                 stop=True)
                nc.vector.tensor_tensor(off_sb[:], off_sb[:], cs_ps[:], ALU.add)

                posk = rt.tile([P, K], F32, tag="posk")
                for k in range(K):
                    tmp = rt.tile([P, E], F32, tag="ttmp2")
                    nc.vector.tensor_tensor(tmp[:], pos[:], ohs[k][:], ALU.mult)
                    nc.vector.tensor_reduce(posk[:, k:k + 1], tmp[:], axis=AX.X,
                                            op=ALU.add)
                ekf = rt.tile([P, K], F32, tag="ekf")
                nc.vector.tensor_copy(ekf[:], i8[:, 0:K])

                ssum = rt.tile([P, 1], F32, tag="ssum")
                nc.vector.tensor_reduce(ssum[:], scok[:], axis=AX.X, op=ALU.add)
                nc.vector.tensor_scalar(ssum[:], ssum[:], 1e-20, None, op0=ALU.add)
                sinv = rt.tile([P, 1], F32, tag="sinv")
                nc.vector.reciprocal(sinv[:], ssum[:])
                nc.vector.tensor_scalar(sinv[:], sinv[:], RSF, None, op0=ALU.mult)
                wkt = wk_keep[:, it * K:(it + 1) * K]
                nc.vector.tensor_scalar(wkt, scok[:], sinv[:], None, op0=ALU.mult)
                keep = rt.tile([P, K], F32, tag="keep")
                nc.vector.tensor_scalar(keep[:], posk[:], C - 0.5, None,
                                        op0=ALU.is_lt)
                nc.vector.tensor_tensor(wkt, wkt, keep[:], ALU.mult)
                nc.vector.tensor_scalar(keep[:], ekf[:], float(EL) - 0.5, None,
                                        op0=ALU.is_lt)
                nc.vector.tensor_tensor(wkt, wkt, keep[:], ALU.mult)

                # local slot index per (t, k): ek*C + posk, clamped into range
                # (non-local / dropped pairs have weight 0; the clamp keeps the
                # gather inside y_buf so it reads finite data)
                slf = rt.tile([P, K], F32, tag="slf")
                nc.vector.tensor_scalar(slf[:], ekf[:], float(C), None,
                                        op0=ALU.mult)
                nc.vector.tensor_tensor(slf[:], slf[:], posk[:], ALU.add)
                nc.vector.tensor_scalar(slf[:], slf[:], float(EL * C - 1), None,
                                        op0=ALU.min)
                nc.vector.tensor_copy(sl_keep[:, it * K:(it + 1) * K], slf[:])

                # slot table via PE: P_all[t, (e, c)] = [pos[t, e] == c][A=1]
                # for local experts e < EL; table = sum_t t * P_all
                pall = tbl_pool.tile([P, EL * C], F32, tag="pall")
                posb = pos[:, 0:EL].rearrange(
                    "p (e o) -> p e o", o=1).broadcast_to((P, EL, C))
                nc.vector.tensor_tensor(
                    pall[:].rearrange("p (e c) -> p e c", e=EL), c_iota[:]
                    .rearrange("p (e c) -> p e c", e=EL), posb, ALU.is_equal)
                ab = A[:, 0:EL].rearrange(
                    "p (e o) -> p e o", o=1).broadcast_to((P, EL, C))
                nc.vector.tensor_tensor(
                    pall[:].rearrange("p (e c) -> p e c", e=EL),
                    pall[:].rearrange("p (e c) -> p e c", e=EL), ab, ALU.mult)
                tokcol = rt.tile([P, 1], F32, tag="tokcol")
                nc.gpsimd.iota(tokcol[:], pattern=[[0, 1]], base=it * P,
                               channel_multiplier=1,
                               allow_small_or_imprecise_dtypes=True)
                for cb in range(EL * C // 512):
                    nc.tensor.matmul(tbl_ps[:, cb * 512:(cb + 1) * 512],
                                     tokcol[:], pall[:, cb * 512:(cb + 1) * 512],
                                     start=(it == 0), stop=(it == NT - 1))

            tblf = tbl_pool.tile([1, EL * C], F32, tag="tblf")
            nc.vector.tensor_copy(tblf[:], tbl_ps[:])
            tblu = tbl_pool.tile([1, EL * C], U32, tag="tblu")
            nc.vector.tensor_copy(tblu[:], tblf[:])
            nc.sync.dma_start(table[:], tblu[:])

            r_ctx.close()

            # -------- shared MLP (fp32, TP slice SL=256)
            with ExitStack() as s_ctx:
                sh_pool = s_ctx.enter_context(tc.tile_pool(name="sh", bufs=3))
                sd_pool = s_ctx.enter_context(tc.tile_pool(name="sd", bufs=1))
                ash_pool = s_ctx.enter_context(tc.tile_pool(name="ash", bufs=2))
                hsh_ps_pool = s_ctx.enter_context(
                    tc.tile_pool(name="hshps", bufs=1, space="PSUM"))
                ysh_ps_pool = s_ctx.enter_context(
                    tc.tile_pool(name="yshps", bufs=2, space="PSUM"))
                for tcn in range(2):  # halves of T
                    hsh_ps = hsh_ps_pool.tile([P, 4 * 512], F32, tag="hsh")
                    for kt in range(NKH):
                        sgu_t = sh_pool.tile([P, 2 * SL], F32, tag="sgu")
                        nc.sync.dma_start(sgu_t[:], sgu_loc[kt * P:(kt + 1) * P, :])
                        hv = hst_pool.tile([P, 512], F32, tag="hsts")
                        nc.sync.dma_start(
                            hv[:],
                            hsT[kt * P:(kt + 1) * P, tcn * 512:(tcn + 1) * 512])
                        for mt in range(4):
                            nc.tensor.matmul(
                                hsh_ps[:, mt * 512:(mt + 1) * 512],
                                sgu_t[:, mt * P:(mt + 1) * P],
                                hv[:],
                                start=(kt == 0), stop=(kt == NKH - 1))
                    # silu(gate)*up: m-tiles 0,1 = gate rows; 2,3 = up rows
                    ash_t = ash_pool.tile([P, 2 * 512], F32, tag="ash")
                    for mt in range(2):
                        sil = rt.tile([P, 512], F32, tag="sil")
                        nc.scalar.activation(sil[:],
                                             hsh_ps[:, mt * 512:(mt + 1) * 512],
                                             ACTF.Sigmoid)
                        nc.vector.tensor_tensor(
                            sil[:], sil[:],
                            hsh_ps[:, mt * 512:(mt + 1) * 512], ALU.mult)
                        nc.vector.tensor_tensor(
                            ash_t[:, mt * 512:(mt + 1) * 512], sil[:],
                            hsh_ps[:, (mt + 2) * 512:(mt + 3) * 512], ALU.mult)
                    sd_ts = []
                    for kt in range(2):
                        sd_t = sd_pool.tile([P, H], F32, tag=f"sd{kt}")
                        nc.sync.dma_start(sd_t[:], sd_loc[kt * P:(kt + 1) * P, :])
                        sd_ts.append(sd_t)
                    for ts in range(4):  # token sub-tiles of 128 in this half
                        tglob = tcn * 4 + ts
                        for hc in range(4):
                            ysh_ps = ysh_ps_pool.tile([P, 512], F32, tag="ysh")
                            for kt in range(2):
                                nc.tensor.matmul(
                                    ysh_ps[:],
                                    ash_t[:, kt * 512 + ts * P:
                                          kt * 512 + (ts + 1) * P],
                                    sd_ts[kt][:, hc * 512:(hc + 1) * 512],
                                    start=(kt == 0), stop=(kt == 1))
                            ysb = rt.tile([P, 512], F32, tag="ysb_sh")
                            nc.any.tensor_copy(ysb[:], ysh_ps[:])
                            nc.sync.dma_start(
                                cc_in[tglob * P:(tglob + 1) * P,
                                      hc * 512:(hc + 1) * 512], ysb[:])

        # ============ phase E: expert GEMMs (float32r)
        with ExitStack() as e_ctx:
            xe_pool = e_ctx.enter_context(tc.tile_pool(name="xe", bufs=2))
            xet_pool = e_ctx.enter_context(tc.tile_pool(name="xet", bufs=2))
            at_pool = e_ctx.enter_context(tc.tile_pool(name="at", bufs=2))
            ysb_pool = e_ctx.enter_context(tc.tile_pool(name="ysb", bufs=3))
            ht_ps_pool = e_ctx.enter_context(
                tc.tile_pool(name="htps", bufs=3, space="PSUM"))
            y_ps_pool = e_ctx.enter_context(
                tc.tile_pool(name="yps", bufs=2, space="PSUM"))
            tr_ps_pool = e_ctx.enter_context(
                tc.tile_pool(name="trps", bufs=2, space="PSUM"))

            for e in range(EL):
                # gather this expert's tokens, transpose to [H, C] layout
                xet_t = xet_pool.tile([P, NKH * 256], F32R, tag="xet")
                for ct in range(2):
                    idxt = rt.tile([P, 1], U32, tag="idxt")
                    nc.sync.dma_start(
                        idxt[:], table[e * C + ct * P: e * C + (ct + 1) * P, :])
                    xe_t = xe_pool.tile([P, H], F32R, tag="xe")
                    nc.gpsimd.indirect_dma_start(
                        xe_t[:], None, hs_r[:],
                        bass.IndirectOffsetOnAxis(ap=idxt[:], axis=0),
                    )
                    for kt in range(NKH):
                        tr_ps = tr_ps_pool.tile([P, P], F32R, tag="trps")
                        nc.tensor.transpose(tr_ps[:], xe_t[:, kt * P:(kt + 1) * P],
                                            id_t[:])
                        nc.vector.tensor_copy(
                            xet_t[:, kt * 256 + ct * P: kt * 256 + (ct + 1) * P],
                            tr_ps[:])

                # gate_up with (gate m, up m+8) pairing for silu_mul
                at_t = at_pool.tile([P, NIT * 256], F32R, tag="at")
                for mp in range(NIT):
                    hts = []
                    for m in (mp, mp + NIT):
                        w13_t = we_pool.tile([P, NKH * P], F32R, tag="w13")
                        nc.sync.dma_start(
                            w13_t[:].rearrange("p (kt m) -> p kt m", kt=NKH),
                            w13_loc[e, :, m * P:(m + 1) * P].rearrange(
                                "(kt p) m -> p kt m", p=P))
                        ht_ps = ht_ps_pool.tile([P, 256], F32, tag="htps")
                        for kt in range(NKH):
                            nc.tensor.matmul(
                                ht_ps[:],
                                w13_t[:, kt * P:(kt + 1) * P],
                                xet_t[:, kt * 256:(kt + 1) * 256],
                                start=(kt == 0), stop=(kt == NKH - 1))
                        hts.append(ht_ps)
                    sil = rt.tile([P, 256], F32, tag="sil_e")
                    nc.scalar.activation(sil[:], hts[0][:], ACTF.Sigmoid)
                    nc.vector.tensor_tensor(sil[:], sil[:], hts[0][:], ALU.mult)
                    nc.vector.tensor_tensor(at_t[:, mp * 256:(mp + 1) * 256],
                                            sil[:], hts[1][:], ALU.mult)

                # down proj; w2 half-chunks loaded once per hc, used by both
                # C-tiles; y written back chunk-wise
                for hc in range(4):
                    w2h = []
                    for ih in range(2):
                        w2_t = w2_pool.tile([P, 4 * 512], F32R, tag="w2")
                        nc.sync.dma_start(
                            w2_t[:].rearrange("p (it n) -> p it n", it=4),
                            w2_loc[e, ih * 512:(ih + 1) * 512,
                                   hc * 512:(hc + 1) * 512].rearrange(
                                "(it p) n -> p it n", p=P))
                        w2h.append(w2_t)
                    for ct in range(2):
                        y_ps = y_ps_pool.tile([P, 512], F32, tag="yps")
                        for it in range(NIT):
                            nc.tensor.matmul(
                                y_ps[:],
                                at_t[:, it * 256 + ct * P: it * 256 + (ct + 1) * P],
                                w2h[it // 4][:, (it % 4) * 512:(it % 4 + 1) * 512],
                                start=(it == 0), stop=(it == NIT - 1))
                        ysb_t = ysb_pool.tile([P, 512], F32, tag="ysb")
                        nc.any.tensor_copy(ysb_t[:], y_ps[:])
                        nc.sync.dma_start(
                            y_buf[e * C + ct * P: e * C + (ct + 1) * P,
                                  hc * 512:(hc + 1) * 512], ysb_t[:])

        # ============ phase C: combine with routing weights
        with ExitStack() as c_ctx:
            cmb_pool = c_ctx.enter_context(tc.tile_pool(name="cmb", bufs=2))
            acc_pool = c_ctx.enter_context(tc.tile_pool(name="acc", bufs=2))
            for it in range(NT):
                acc = acc_pool.tile([P, H], F32, tag="acc")
                nc.sync.dma_start(acc[:], cc_in[it * P:(it + 1) * P, :])
                for k in range(K):
                    yg = cmb_pool.tile([P, H], F32, tag="yg")
                    nc.gpsimd.indirect_dma_start(
                        yg[:], None, y_buf[:],
                        bass.IndirectOffsetOnAxis(
                            ap=sl_keep[:, it * K + k: it * K + k + 1], axis=0),
                    )
                    nc.vector.scalar_tensor_tensor(
                        acc[:], yg[:],
                        wk_keep[:, it * K + k: it * K + k + 1],
                        acc[:], op0=ALU.mult, op1=ALU.add)
                nc.sync.dma_start(cc_in[it * P:(it + 1) * P, :], acc[:])

        # ============ cross-core reduce
        if use_collective:
            nc.gpsimd.collective_compute(
                "ReduceScatter", ALU.add,
                replica_groups=[list(range(NCORES))],
                ins=[cc_in[:]],
                outs=[cc_out[:]],
            )
            nc.sync.dma_start(out_t[:], cc_out[:])
        else:
            nc.sync.dma_start(out_t[:], cc_in[:])

    nc.compile()
    return nc


def make_in_maps(inputs):
    hs = np.ascontiguousarray(np.asarray(inputs["hidden_states"], np.float32))
    gate_w = np.asarray(inputs["gate_w"], np.float32)
    gate_bias = np.asarray(inputs["gate_bias"], np.float32)
    w13 = np.asarray(inputs["w13"], np.float32)
    w2 = np.asarray(inputs["w2"], np.float32)
    sgu = np.asarray(inputs["shared_gate_up"], np.float32)
    sd = np.asarray(inputs["shared_down"], np.float32)

    hsT = np.ascontiguousarray(hs.T)
    gwT = np.ascontiguousarray(gate_w.T)
    bias_bc = np.ascontiguousarray(np.broadcast_to(gate_bias, (P, E)))
    u128 = (np.arange(P)[:, None] < np.arange(P)[None, :]).astype(np.float32)
    ones1 = np.ones((1, P), np.float32)
    ident = np.eye(P, dtype=np.float32)

    in_maps = []
    for c in range(NCORES):
        # group permutation: core c's expert group lands at columns 0..EL-1 so
        # "local expert" logic is compile-time static; grouped top-k is
        # invariant to reordering whole groups
        gperm = [c] + [g for g in range(G) if g != c]
        eperm = np.concatenate([np.arange(g * EL, (g + 1) * EL) for g in gperm])
        sgu_c = np.ascontiguousarray(
            np.concatenate([sgu[:, c * SL:(c + 1) * SL],
                            sgu[:, S + c * SL:S + (c + 1) * SL]], axis=1))
        in_maps.append({
            "hs_r": hs,
            "hsT": hsT,
            "gwT": np.ascontiguousarray(gwT[:, eperm]),
            "bias_bc": np.ascontiguousarray(bias_bc[:, eperm]),
            "u128": u128,
            "ones1": ones1,
            "onesc": np.ones((P, 1), np.float32),
            "ident_r": ident,
            "w13_loc": np.ascontiguousarray(w13[c * EL:(c + 1) * EL]),
            "w2_loc": np.ascontiguousarray(w2[c * EL:(c + 1) * EL]),
            "sgu_loc": sgu_c,
            "sd_loc": np.ascontiguousarray(sd[c * SL:(c + 1) * SL]),
        })
    return in_maps


def kernel(**inputs):
    nc = build_program(use_collective=True)
    in_maps = make_in_maps(inputs)
    res = bass_utils.run_bass_kernel_spmd(nc, in_maps,
                                          core_ids=list(range(NCORES)))
    return np.concatenate([res.results[c]["out"] for c in range(NCORES)], axis=0)


# revision 21
# speedup vs baseline: 3.7871x; 3.7871x over previous
"""DeepSeek-V2-Lite-style MoE layer on 8 Trainium2 NeuronCores.

Sharding: expert-parallel (8 experts/core) for the routed experts; the shared
MLP is tensor-parallel along its intermediate dim (256/core); the fp32 router
and grouped-top-k routing run replicated on every core. Each core gathers the
tokens routed to its local experts straight from (replicated) hidden_states,
runs the grouped GEMMs in float32r, combines locally with the routing weights
plus its shared-MLP partial, and a ReduceScatter sums partials across cores so
core c returns output rows [128c, 128(c+1)).

Config (DeepSeek-V2-Lite): T=1024 H=2048 E=64 K=6 I=1024 G=8 TG=3 C=256
"""
import numpy as np
from contextlib import ExitStack

import concourse.bass as bass
import concourse.tile as tile
from concourse import bacc
from concourse import mybir
from concourse import bass_utils

T, H, E, K, I, G, TG, C = 1024, 2048, 64, 6, 1024, 8, 3, 256
S = 2048                  # full shared intermediate (NS * I)
RSF = 2.5
NCORES = 8
EL = E // NCORES          # local experts per core
SL = S // NCORES          # local shared-intermediate slice per core
P = 128
NT = T // P               # token tiles
NKH = H // P              # K-tiles over H
NIT = I // P              # I-tiles
BIG = 1.0e30

F32 = mybir.dt.float32
F32R = mybir.dt.float32r
U32 = mybir.dt.uint32
BF16 = mybir.dt.bfloat16
U8 = mybir.dt.uint8
AX = mybir.AxisListType
ALU = mybir.AluOpType
ACTF = mybir.ActivationFunctionType


def build_program(use_collective=True):
    nc = bacc.Bacc("TRN2", target_bir_lowering=False, debug=False,
                   num_devices=NCORES)

    # ---- inputs (per-core; sliced/prepped on host)
    hs_bf = nc.dram_tensor("hs_bf", [T, H], BF16, kind="ExternalInput")
    hsT = nc.dram_tensor("hsT", [H, T], F32, kind="ExternalInput")
    hsT_bf = nc.dram_tensor("hsT_bf", [H, T], BF16, kind="ExternalInput")
    gwT = nc.dram_tensor("gwT", [H, E], F32, kind="ExternalInput")
    bias_bc = nc.dram_tensor("bias_bc", [P, E], F32, kind="ExternalInput")
    u128 = nc.dram_tensor("u128", [P, P], F32, kind="ExternalInput")
    ones1 = nc.dram_tensor("ones1", [1, P], F32, kind="ExternalInput")
    onesc = nc.dram_tensor("onesc", [P, 1], F32, kind="ExternalInput")
    ident_b = nc.dram_tensor("ident_b", [P, P], BF16, kind="ExternalInput")
    # w13 packed on host: [EL, NIT, H, 256] with last dim = concat(gate
    # m-slice, up m-slice) so each (e, mp) chunk is one contiguous DMA
    w13_loc = nc.dram_tensor("w13_loc", [EL, NIT, H, 256], BF16,
                             kind="ExternalInput")
    w2_loc = nc.dram_tensor("w2_loc", [EL, I, H], BF16, kind="ExternalInput")
    sgu_loc = nc.dram_tensor("sgu_loc", [H, 2 * SL], BF16, kind="ExternalInput")
    sd_loc = nc.dram_tensor("sd_loc", [SL, H], BF16, kind="ExternalInput")

    out_t = nc.dram_tensor("out", [P if use_collective else T, H], F32,
                           kind="ExternalOutput")

    # ---- internal DRAM scratch
    table = nc.dram_tensor("table", [EL * C, 1], U32, kind="Internal")
    y_buf = nc.dram_tensor("y_buf", [EL * C, H], BF16, kind="Internal")
    cc_in = nc.dram_tensor("cc_in", [T, H], F32, kind="Internal")
    if use_collective:
        cc_out = nc.dram_tensor("cc_out", [P, H], F32, kind="Internal")

    with tile.TileContext(nc) as tc, ExitStack() as ctx:
        cst = ctx.enter_context(tc.tile_pool(name="cst", bufs=1))
        rt = ctx.enter_context(tc.tile_pool(name="rt", bufs=2))
        rkeep = ctx.enter_context(tc.tile_pool(name="rkeep", bufs=1))
        # expert weight pools open for the whole kernel so their DMAs can
        # prefetch during routing/shared phases
        we_pool = ctx.enter_context(tc.tile_pool(name="we", bufs=3))
        w2_pool = ctx.enter_context(tc.tile_pool(name="w2", bufs=2))

        # ---------------- constants
        bias_t = cst.tile([P, E], F32, tag="bias")
        nc.sync.dma_start(bias_t[:], bias_bc[:])
        u_t = cst.tile([P, P], F32, tag="u128")
        nc.sync.dma_start(u_t[:], u128[:])
        ones_t = cst.tile([1, P], F32, tag="ones1")
        nc.sync.dma_start(ones_t[:], ones1[:])
        onesc_t = cst.tile([P, 1], F32, tag="onesc")
        nc.sync.dma_start(onesc_t[:], onesc[:])
        id_t = cst.tile([P, P], BF16, tag="ident")
        nc.sync.dma_start(id_t[:], ident_b[:])
        gw_t = cst.tile([P, NKH * E], F32, tag="gwT")   # [128, kt*64]
        nc.sync.dma_start(gw_t[:].rearrange("p (kt e) -> p kt e", kt=NKH),
                          gwT[:].rearrange("(kt p) e -> p kt e", p=P))
        iot_e = cst.tile([P, E], U32, tag="iote")
        nc.gpsimd.iota(iot_e[:], pattern=[[1, E]], base=0, channel_multiplier=0)
        # per-slot column index 0..C-1 repeated for the EL local experts
        c_iota = cst.tile([P, EL * C], F32, tag="c_iota")
        nc.gpsimd.iota(c_iota[:], pattern=[[0, EL], [1, C]], base=0,
                       channel_multiplier=0,
                       allow_small_or_imprecise_dtypes=True)
        off_sb = cst.tile([1, E], F32, tag="off")
        nc.vector.memset(off_sb[:], 0.0)

        # per-(token,k) combine data, resident until the combine phase
        wk_keep = rkeep.tile([P, NT * K], F32, tag="wk_keep")
        sl_keep = rkeep.tile([P, NT * K], U32, tag="sl_keep")

        # ============ phases R (routing) + S (shared MLP): own scratch pools
        with ExitStack() as rs_ctx:
            hst_pool = rs_ctx.enter_context(tc.tile_pool(name="hst", bufs=4))
            tbl_pool = rs_ctx.enter_context(tc.tile_pool(name="tblsb", bufs=1))
            r_ctx = rs_ctx.enter_context(ExitStack())
            lg_ps_pool = r_ctx.enter_context(
                tc.tile_pool(name="lgps", bufs=2, space="PSUM"))
            pos_ps_pool = r_ctx.enter_context(
                tc.tile_pool(name="posps", bufs=1, space="PSUM"))
            cs_ps_pool = r_ctx.enter_context(
                tc.tile_pool(name="csps", bufs=1, space="PSUM"))

            # -------- router matmul: stream [128,128] hsT slices per (tt, kt)
            logits_list = []
            for tt in range(NT):
                lg = lg_ps_pool.tile([P, E], F32, tag="logits")
                for kt in range(NKH):
                    hv = hst_pool.tile([P, P], F32, tag="hstr")
                    nc.sync.dma_start(
                        hv[:], hsT[kt * P:(kt + 1) * P, tt * P:(tt + 1) * P])
                    nc.tensor.matmul(
                        lg[:], hv[:],
                        gw_t[:, kt * E:(kt + 1) * E],
                        start=(kt == 0), stop=(kt == NKH - 1))
                logits_list.append(lg)

            # -------- grouped top-k routing per token tile
            tbl_ps = cs_ps_pool.tile([1, EL * C], F32, tag="tblps")
            for it in range(NT):
                scores = rt.tile([P, E], F32, tag="scores")
                nc.scalar.activation(scores[:], logits_list[it][:],
                                     ACTF.Sigmoid)
                sc = rt.tile([P, E], F32, tag="sc")
                nc.vector.tensor_tensor(sc[:], scores[:], bias_t[:], ALU.add)

                grp = sc[:].rearrange("p (g e) -> p g e", g=G)
                m1 = rt.tile([P, G], F32, tag="m1")
                nc.vector.tensor_reduce(m1[:], grp, axis=AX.X, op=ALU.max)
                eq = rt.tile([P, E], F32, tag="eq")
                m1b = m1[:].rearrange("p (g o) -> p g o", o=1).broadcast_to((P, G, G))
                nc.vector.tensor_tensor(eq[:].rearrange("p (g e) -> p g e", g=G),
                                        grp, m1b, ALU.is_ge)
                pen = rt.tile([P, E], F32, tag="pen")
                nc.vector.tensor_scalar(pen[:], eq[:], -BIG, None, op0=ALU.mult)
                msk2 = rt.tile([P, E], F32, tag="msk2")
                nc.vector.tensor_tensor(msk2[:], sc[:], pen[:], ALU.add)
                m2 = rt.tile([P, G], F32, tag="m2")
                nc.vector.tensor_reduce(
                    m2[:], msk2[:].rearrange("p (g e) -> p g e", g=G),
                    axis=AX.X, op=ALU.max)
                g2 = rt.tile([P, G], F32, tag="g2")
                nc.vector.tensor_tensor(g2[:], m1[:], m2[:], ALU.add)

                gv8 = rt.tile([P, 8], F32, tag="gv8")
                gi8 = rt.tile([P, 8], U32, tag="gi8")
                nc.vector.max_with_indices(gv8[:], gi8[:], g2[:])
                gmask = rt.tile([P, G], F32, tag="gmask")
                nc.vector.tensor_tensor(gmask[:], g2[:],
                                        gv8[:, 2:3].broadcast_to((P, G)), ALU.is_ge)

                gm64 = rt.tile([P, E], U8, tag="gm64")
                gmb = gmask[:].rearrange("p (g o) -> p g o", o=1) \
                    .broadcast_to((P, G, G))
                nc.vector.tensor_copy(gm64[:].rearrange("p (g e) -> p g e", g=G),
                                      gmb)
                scm = rt.tile([P, E], F32, tag="scm")
                nc.vector.memset(scm[:], -BIG)
                nc.vector.copy_predicated(scm[:], gm64[:], sc[:])
                v8 = rt.tile([P, 8], F32, tag="v8")
                i8 = rt.tile([P, 8], U32, tag="i8")
                nc.vector.max_with_indices(v8[:], i8[:], scm[:])

                A = rt.tile([P, E], F32, tag="A")
                nc.vector.memset(A[:], 0.0)
                scok = rt.tile([P, K], F32, tag="scok")
                ohs = []
                for k in range(K):
                    oh = rt.tile([P, E], F32, tag=f"oh{k}")
                    ohs.append(oh)
                    nc.vector.tensor_tensor(oh[:], iot_e[:],
                                            i8[:, k:k + 1].broadcast_to((P, E)),
                                            ALU.is_equal)
                    nc.vector.tensor_tensor(A[:], A[:], oh[:], ALU.add)
                    tmp = rt.tile([P, E], F32, tag="ttmp")
                    nc.vector.tensor_tensor(tmp[:], scores[:], oh[:], ALU.mult)
                    nc.vector.tensor_reduce(scok[:, k:k + 1], tmp[:], axis=AX.X,
                                            op=ALU.add)

                pos_ps = pos_ps_pool.tile([P, E], F32, tag="posps")
                nc.tensor.matmul(pos_ps[:], u_t[:], A[:], start=True, stop=False)
                nc.tensor.matmul(pos_ps[:], ones_t[:], off_sb[:], start=False,
                                 stop=True)
                pos = rt.tile([P, E], F32, tag="pos")
                nc.vector.tensor_copy(pos[:], pos_ps[:])
                cs_ps = cs_ps_pool.tile([1, E], F32, tag="csps")
                nc.tensor.matmul(cs_ps[:], onesc_t[:], A[:], start=True,
                                 stop=True)
                nc.vector.tensor_tensor(off_sb[:], off_sb[:], cs_ps[:], ALU.add)

                posk = rt.tile([P, K], F32, tag="posk")
                for k in range(K):
                    tmp = rt.tile([P, E], F32, tag="ttmp2")
                    nc.vector.tensor_tensor(tmp[:], pos[:], ohs[k][:], ALU.mult)
                    nc.vector.tensor_reduce(posk[:, k:k + 1], tmp[:], axis=AX.X,
                                            op=ALU.add)
                ekf = rt.tile([P, K], F32, tag="ekf")
                nc.vector.tensor_copy(ekf[:], i8[:, 0:K])

                ssum = rt.tile([P, 1], F32, tag="ssum")
                nc.vector.tensor_reduce(ssum[:], scok[:], axis=AX.X, op=ALU.add)
                nc.vector.tensor_scalar(ssum[:], ssum[:], 1e-20, None, op0=ALU.add)
                sinv = rt.tile([P, 1], F32, tag="sinv")
                nc.vector.reciprocal(sinv[:], ssum[:])
                nc.vector.tensor_scalar(sinv[:], sinv[:], RSF, None, op0=ALU.mult)
                wkt = wk_keep[:, it * K:(it + 1) * K]
                nc.vector.tensor_scalar(wkt, scok[:], sinv[:], None, op0=ALU.mult)
                keep = rt.tile([P, K], F32, tag="keep")
                nc.vector.tensor_scalar(keep[:], posk[:], C - 0.5, None,
                                        op0=ALU.is_lt)
                nc.vector.tensor_tensor(wkt, wkt, keep[:], ALU.mult)
                nc.vector.tensor_scalar(keep[:], ekf[:], float(EL) - 0.5, None,
                                        op0=ALU.is_lt)
                nc.vector.tensor_tensor(wkt, wkt, keep[:], ALU.mult)

                # local slot index per (t, k): ek*C + posk, clamped into range
                # (non-local / dropped pairs have weight 0; the clamp keeps the
                # gather inside y_buf so it reads finite data)
                slf = rt.tile([P, K], F32, tag="slf")
                nc.vector.tensor_scalar(slf[:], ekf[:], float(C), None,
                                        op0=ALU.mult)
                nc.vector.tensor_tensor(slf[:], slf[:], posk[:], ALU.add)
                nc.vector.tensor_scalar(slf[:], slf[:], float(EL * C - 1), None,
                                        op0=ALU.min)
                nc.vector.tensor_copy(sl_keep[:, it * K:(it + 1) * K], slf[:])

                # slot table via PE: P_all[t, (e, c)] = [pos[t, e] == c][A=1]
                # for local experts e < EL; table = sum_t t * P_all
                pall = tbl_pool.tile([P, EL * C], F32, tag="pall")
                posb = pos[:, 0:EL].rearrange(
                    "p (e o) -> p e o", o=1).broadcast_to((P, EL, C))
                nc.vector.tensor_tensor(
                    pall[:].rearrange("p (e c) -> p e c", e=EL), c_iota[:]
                    .rearrange("p (e c) -> p e c", e=EL), posb, ALU.is_equal)
                ab = A[:, 0:EL].rearrange(
                    "p (e o) -> p e o", o=1).broadcast_to((P, EL, C))
                nc.vector.tensor_tensor(
                    pall[:].rearrange("p (e c) -> p e c", e=EL),
                    pall[:].rearrange("p (e c) -> p e c", e=EL), ab, ALU.mult)
                tokcol = rt.tile([P, 1], F32, tag="tokcol")
                nc.gpsimd.iota(tokcol[:], pattern=[[0, 1]], base=it * P,
                               channel_multiplier=1,
                               allow_small_or_imprecise_dtypes=True)
                for cb in range(EL * C // 512):
                    nc.tensor.matmul(tbl_ps[:, cb * 512:(cb + 1) * 512],
                                     tokcol[:], pall[:, cb * 512:(cb + 1) * 512],
                                     start=(it == 0), stop=(it == NT - 1))

            tblf = tbl_pool.tile([1, EL * C], F32, tag="tblf")
            nc.vector.tensor_copy(tblf[:], tbl_ps[:])
            tblu = tbl_pool.tile([1, EL * C], U32, tag="tblu")
            nc.vector.tensor_copy(tblu[:], tblf[:])
            nc.sync.dma_start(table[:], tblu[:])

            r_ctx.close()

            # -------- shared MLP (fp32, TP slice SL=256)
            with ExitStack() as s_ctx:
                sh_pool = s_ctx.enter_context(tc.tile_pool(name="sh", bufs=3))
                sd_pool = s_ctx.enter_context(tc.tile_pool(name="sd", bufs=1))
                ash_pool = s_ctx.enter_context(tc.tile_pool(name="ash", bufs=2))
                hsh_ps_pool = s_ctx.enter_context(
                    tc.tile_pool(name="hshps", bufs=1, space="PSUM"))
                ysh_ps_pool = s_ctx.enter_context(
                    tc.tile_pool(name="yshps", bufs=2, space="PSUM"))
                for tcn in range(2):  # halves of T
                    hsh_ps = hsh_ps_pool.tile([P, 4 * 512], F32, tag="hsh")
                    for kt in range(NKH):
                        sgu_t = sh_pool.tile([P, 2 * SL], BF16, tag="sgu")
                        nc.sync.dma_start(sgu_t[:], sgu_loc[kt * P:(kt + 1) * P, :])
                        hv = hst_pool.tile([P, 512], BF16, tag="hsts")
                        nc.sync.dma_start(
                            hv[:],
                            hsT_bf[kt * P:(kt + 1) * P, tcn * 512:(tcn + 1) * 512])
                        for mt in range(4):
                            nc.tensor.matmul(
                                hsh_ps[:, mt * 512:(mt + 1) * 512],
                                sgu_t[:, mt * P:(mt + 1) * P],
                                hv[:],
                                start=(kt == 0), stop=(kt == NKH - 1))
                    # silu(gate)*up: m-tiles 0,1 = gate rows; 2,3 = up rows
                    ash_t = ash_pool.tile([P, 2 * 512], BF16, tag="ash")
                    for mt in range(2):
                        sil = rt.tile([P, 512], F32, tag="sil")
                        nc.scalar.activation(sil[:],
                                             hsh_ps[:, mt * 512:(mt + 1) * 512],
                                             ACTF.Sigmoid)
                        nc.vector.tensor_tensor(
                            sil[:], sil[:],
                            hsh_ps[:, mt * 512:(mt + 1) * 512], ALU.mult)
                        nc.vector.tensor_tensor(
                            ash_t[:, mt * 512:(mt + 1) * 512], sil[:],
                            hsh_ps[:, (mt + 2) * 512:(mt + 3) * 512], ALU.mult)
                    sd_ts = []
                    for kt in range(2):
                        sd_t = sd_pool.tile([P, H], BF16, tag=f"sd{kt}")
                        nc.sync.dma_start(sd_t[:], sd_loc[kt * P:(kt + 1) * P, :])
                        sd_ts.append(sd_t)
                    for ts in range(4):  # token sub-tiles of 128 in this half
                        tglob = tcn * 4 + ts
                        for hc in range(4):
                            ysh_ps = ysh_ps_pool.tile([P, 512], F32, tag="ysh")
                            for kt in range(2):
                                nc.tensor.matmul(
                                    ysh_ps[:],
                                    ash_t[:, kt * 512 + ts * P:
                                          kt * 512 + (ts + 1) * P],
                                    sd_ts[kt][:, hc * 512:(hc + 1) * 512],
                                    start=(kt == 0), stop=(kt == 1))
                            ysb = rt.tile([P, 512], F32, tag="ysb_sh")
                            nc.any.tensor_copy(ysb[:], ysh_ps[:])
                            nc.sync.dma_start(
                                cc_in[tglob * P:(tglob + 1) * P,
                                      hc * 512:(hc + 1) * 512], ysb[:])

        # ============ phase E: expert GEMMs (float32r)
        with ExitStack() as e_ctx:
            xe_pool = e_ctx.enter_context(tc.tile_pool(name="xe", bufs=2))
            xet_pool = e_ctx.enter_context(tc.tile_pool(name="xet", bufs=2))
            at_pool = e_ctx.enter_context(tc.tile_pool(name="at", bufs=2))
            ysb_pool = e_ctx.enter_context(tc.tile_pool(name="ysb", bufs=3))
            ht_ps_pool = e_ctx.enter_context(
                tc.tile_pool(name="htps", bufs=3, space="PSUM"))
            y_ps_pool = e_ctx.enter_context(
                tc.tile_pool(name="yps", bufs=2, space="PSUM"))
            tr_ps_pool = e_ctx.enter_context(
                tc.tile_pool(name="trps", bufs=2, space="PSUM"))

            for e in range(EL):
                # gather this expert's tokens, transpose to [H, C] layout
                xet_t = xet_pool.tile([P, NKH * 256], BF16, tag="xet")
                for ct in range(2):
                    idxt = rt.tile([P, 1], U32, tag="idxt")
                    nc.sync.dma_start(
                        idxt[:], table[e * C + ct * P: e * C + (ct + 1) * P, :])
                    xe_t = xe_pool.tile([P, H], BF16, tag="xe")
                    nc.gpsimd.indirect_dma_start(
                        xe_t[:], None, hs_bf[:],
                        bass.IndirectOffsetOnAxis(ap=idxt[:], axis=0),
                    )
                    for kt in range(NKH):
                        tr_ps = tr_ps_pool.tile([P, P], BF16, tag="trps")
                        nc.tensor.transpose(tr_ps[:], xe_t[:, kt * P:(kt + 1) * P],
                                            id_t[:])
                        nc.vector.tensor_copy(
                            xet_t[:, kt * 256 + ct * P: kt * 256 + (ct + 1) * P],
                            tr_ps[:])

                # gate_up: one packed 1MB DMA per (e, mp); gate and up
                # accumulate in two PSUM groups, then silu_mul
                at_t = at_pool.tile([P, NIT * 256], BF16, tag="at")
                for mp in range(NIT):
                    w13_t = we_pool.tile([P, NKH * 256], BF16, tag="w13")
                    nc.sync.dma_start(
                        w13_t[:].rearrange("p (kt j) -> p kt j", kt=NKH),
                        w13_loc[e, mp].rearrange("(kt p) j -> p kt j", p=P))
                    hts = []
                    for half in range(2):
                        ht_ps = ht_ps_pool.tile([P, 256], F32, tag="htps")
                        for kt in range(NKH):
                            nc.tensor.matmul(
                                ht_ps[:],
                                w13_t[:, kt * 256 + half * P:
                                      kt * 256 + (half + 1) * P],
                                xet_t[:, kt * 256:(kt + 1) * 256],
                                start=(kt == 0), stop=(kt == NKH - 1))
                        hts.append(ht_ps)
                    sil = rt.tile([P, 256], F32, tag="sil_e")
                    nc.scalar.activation(sil[:], hts[0][:], ACTF.Sigmoid)
                    nc.vector.tensor_tensor(sil[:], sil[:], hts[0][:], ALU.mult)
                    nc.vector.tensor_tensor(at_t[:, mp * 256:(mp + 1) * 256],
                                            sil[:], hts[1][:], ALU.mult)

                # down proj; w2 half-chunks loaded once per hc, used by both
                # C-tiles; y written back chunk-wise
                for hc in range(4):
                    w2h = []
                    for ih in range(2):
                        w2_t = w2_pool.tile([P, 4 * 512], BF16, tag="w2")
                        nc.sync.dma_start(
                            w2_t[:].rearrange("p (it n) -> p it n", it=4),
                            w2_loc[e, ih * 512:(ih + 1) * 512,
                                   hc * 512:(hc + 1) * 512].rearrange(
                                "(it p) n -> p it n", p=P))
                        w2h.append(w2_t)
                    for ct in range(2):
                        y_ps = y_ps_pool.tile([P, 512], F32, tag="yps")
                        for it in range(NIT):
                            nc.tensor.matmul(
                                y_ps[:],
                                at_t[:, it * 256 + ct * P: it * 256 + (ct + 1) * P],
                                w2h[it // 4][:, (it % 4) * 512:(it % 4 + 1) * 512],
                                start=(it == 0), stop=(it == NIT - 1))
                        ysb_t = ysb_pool.tile([P, 512], BF16, tag="ysb")
                        nc.any.tensor_copy(ysb_t[:], y_ps[:])
                        nc.sync.dma_start(
                            y_buf[e * C + ct * P: e * C + (ct + 1) * P,
                                  hc * 512:(hc + 1) * 512], ysb_t[:])

        # ============ phase C: combine with routing weights
        with ExitStack() as c_ctx:
            cmb_pool = c_ctx.enter_context(tc.tile_pool(name="cmb", bufs=2))
            acc_pool = c_ctx.enter_context(tc.tile_pool(name="acc", bufs=2))
            for it in range(NT):
                acc = acc_pool.tile([P, H], F32, tag="acc")
                nc.sync.dma_start(acc[:], cc_in[it * P:(it + 1) * P, :])
                for k in range(K):
                    yg = cmb_pool.tile([P, H], BF16, tag="yg")
                    nc.gpsimd.indirect_dma_start(
                        yg[:], None, y_buf[:],
                        bass.IndirectOffsetOnAxis(
                            ap=sl_keep[:, it * K + k: it * K + k + 1], axis=0),
                    )
                    nc.vector.scalar_tensor_tensor(
                        acc[:], yg[:],
                        wk_keep[:, it * K + k: it * K + k + 1],
                        acc[:], op0=ALU.mult, op1=ALU.add)
                nc.sync.dma_start(cc_in[it * P:(it + 1) * P, :], acc[:])

        # ============ cross-core reduce
        if use_collective:
            nc.gpsimd.collective_compute(
                "ReduceScatter", ALU.add,
                replica_groups=[list(range(NCORES))],
                ins=[cc_in[:]],
                outs=[cc_out[:]],
            )
            nc.sync.dma_start(out_t[:], cc_out[:])
        else:
            nc.sync.dma_start(out_t[:], cc_in[:])

    nc.compile()
    return nc


def make_in_maps(inputs):
    hs = np.ascontiguousarray(np.asarray(inputs["hidden_states"], np.float32))
    gate_w = np.asarray(inputs["gate_w"], np.float32)
    gate_bias = np.asarray(inputs["gate_bias"], np.float32)
    w13 = np.asarray(inputs["w13"], np.float32)
    w2 = np.asarray(inputs["w2"], np.float32)
    sgu = np.asarray(inputs["shared_gate_up"], np.float32)
    sd = np.asarray(inputs["shared_down"], np.float32)

    import ml_dtypes
    bf16 = ml_dtypes.bfloat16
    hsT = np.ascontiguousarray(hs.T)
    gwT = np.ascontiguousarray(gate_w.T)
    bias_bc = np.ascontiguousarray(np.broadcast_to(gate_bias, (P, E)))
    u128 = (np.arange(P)[:, None] < np.arange(P)[None, :]).astype(np.float32)
    ones1 = np.ones((1, P), np.float32)
    ident = np.eye(P, dtype=bf16)
    hs_bf = hs.astype(bf16)
    hsT_bf = hsT.astype(bf16)

    in_maps = []
    for c in range(NCORES):
        # group permutation: core c's expert group lands at columns 0..EL-1 so
        # "local expert" logic is compile-time static; grouped top-k is
        # invariant to reordering whole groups
        gperm = [c] + [g for g in range(G) if g != c]
        eperm = np.concatenate([np.arange(g * EL, (g + 1) * EL) for g in gperm])
        sgu_c = np.ascontiguousarray(
            np.concatenate([sgu[:, c * SL:(c + 1) * SL],
                            sgu[:, S + c * SL:S + (c + 1) * SL]], axis=1))
        wl = w13[c * EL:(c + 1) * EL]
        w13g = wl[:, :, :I].reshape(EL, H, NIT, P)
        w13u = wl[:, :, I:].reshape(EL, H, NIT, P)
        w13p = np.concatenate([w13g, w13u], axis=-1).transpose(0, 2, 1, 3)
        in_maps.append({
            "hs_bf": hs_bf,
            "hsT": hsT,
            "hsT_bf": hsT_bf,
            "gwT": np.ascontiguousarray(gwT[:, eperm]),
            "bias_bc": np.ascontiguousarray(bias_bc[:, eperm]),
            "u128": u128,
            "ones1": ones1,
            "onesc": np.ones((P, 1), np.float32),
            "ident_b": ident,
            "w13_loc": np.ascontiguousarray(w13p).astype(bf16),
            "w2_loc": w2[c * EL:(c + 1) * EL].astype(bf16),
            "sgu_loc": sgu_c.astype(bf16),
            "sd_loc": np.ascontiguousarray(sd[c * SL:(c + 1) * SL]).astype(bf16),
        })
    return in_maps


def kernel(**inputs):
    nc = build_program(use_collective=True)
    in_maps = make_in_maps(inputs)
    res = bass_utils.run_bass_kernel_spmd(nc, in_maps,
                                          core_ids=list(range(NCORES)))
    return np.concatenate([res.results[c]["out"] for c in range(NCORES)], axis=0)
